# revision 1
# baseline (speedup 1.0000x reference)
"""DeepWalk hierarchical-softmax scoring kernel for 8 Trainium2 NeuronCores.

Computation (mirrors the nn.Module reference):
    path = heap ancestors of leaf u_k           (L ~ 19-20 static ints)
    emd  = emd_weight[v_j]                      [128]
    hv   = hs_weight[path]                      [L, 128]
    out  = -prod(log_sigmoid(hv @ emd))         scalar f32

Distribution: full replication (batch-size-1 degenerate case of the hint's
data parallelism). Both tables are staged whole into every core's HBM as one
concatenated [hs; emd] table; each core runs the complete lookup + score
locally and core 0's scalar is returned. For a single walk this strictly
dominates model-parallel sharding: a cross-core collective costs ~15us flat,
an order of magnitude more than the entire computation.

Per-core dataflow (gather mode "dma_gather" — SWDGE gathers cost a tiny
fraction of a generic DMA here, and the 4-byte result leaves via a sequencer
register store, so the kernel has no DMACopy at all):
  The int16 gather index table is COMPUTED on-device from the heap
  recurrence path[k] = ((leaf+1) >> (k+1)) - 1 via iota + shift (no DMA),
  replicated across all 8 Q7 cores' 16-partition table views. The product
  of logsigmoids is order-invariant, so path rows may land in any partition
  order. Six small hardware gathers fetch the rows:
    G_emd: L replicated copies of the emd row (all-zero index table,
           window base = emd row) -> gev partitions 0..L-1
    G_b(k), k=NB-1..0: the NB path rows whose index exceeds int16 reach,
           each flooding gs partitions 0..15+k with row 16*q_k + c_k from
           a 16-row-strided window (q_k fits int16); the descending chain
           leaves path[k] at partition 15+k
    G_small: the L-NB small-index path rows overwrite partitions 0..L-NB-1
  DVE: one scalar_tensor_tensor -> pd[L,1], the complete dot products
  ACT: ea=Exp(-pd); sp=Ln(ea+1)=softplus(-dots); lt=Ln(sp)
  PE:  ps[1,1] = lt.T @ ones  (sum over the L partitions)
  ACT: res=Exp(ps) = prod(softplus)
  out: sequencer TENSOR_LOAD + TENSOR_SAVE of the 4-byte result (no DMA)
(no softplus in this build's ACT tables; Exp and Ln share the
natural_log_exp_and_others table set, loaded once by an explicit
LoadActFuncSet emitted before the block streams so it hides under the
gathers; the ACT table load is the critical path of the whole kernel)
"""

import contextlib

import numpy as np

import concourse.bass as bass
import concourse.mybir as mybir
from concourse.bass_utils import run_bass_kernel_spmd

NUM_V = 1_000_000
EMD_DIM = 128
N_CORES = 8
F32 = mybir.dt.float32
I32 = mybir.dt.int32
I16 = mybir.dt.int16
TBL_ROWS = 2 * NUM_V - 1  # concat(hs_weight, emd_weight) rows
SMALL_WIN = 32768  # int16 index reach of one dma_gather window

GATHER_MODE = "dma_gather"  # "dma_gather" | "indirect"


def hs_path(u_k: int, num_V: int = NUM_V) -> list[int]:
    """Heap indices of all ancestors of leaf u_k, down-to-root (incl. 0)."""
    n = num_V - 1 + u_k
    path = []
    while n > 0:
        n = (n - 1) // 2
        path.append(n)
    return path


def build_module(v_j: int, u_k: int):
    """Build the per-core Bass module. v_j/u_k are compile-time constants,
    mirroring the reference where the path is a static int array."""
    path = hs_path(u_k)
    L = len(path)
    leaf1 = NUM_V + u_k  # (leaf index + 1): path[k] = (leaf1 >> (k+1)) - 1
    bigs = [p for p in path if p >= SMALL_WIN]
    NB = len(bigs)
    assert bigs == path[:NB] and NB <= 16, (path, bigs)
    assert all(p < SMALL_WIN for p in path[NB:])
    emd_row = (NUM_V - 1) + v_j  # emd_weight[v_j] inside the concat table
    nidx = 32 + L  # indirect mode: 20 path + pad + 20 emd copies
    idx_cols = -(-L // 16)  # int16 idx table columns (16-wrapped)

    if GATHER_MODE == "dma_gather":
        # Bacc (vs raw Bass) for its compile passes: InstISA subclass codegen
        # (dma_gather) and automatic GPSIMD library-load insertion.
        from concourse.bacc import Bacc

        nc = Bacc("TRN2", num_devices=N_CORES)
    else:
        nc = bass.Bass(num_devices=N_CORES)

    tbl = nc.dram_tensor("tbl", [TBL_ROWS, EMD_DIM], F32, kind="ExternalInput")
    out = nc.dram_tensor("out", [1, 1], F32, kind="ExternalOutput")

    ctx = contextlib.ExitStack()
    with ctx:
        pd = ctx.enter_context(nc.sbuf_tensor("pd", [L, 1], F32))
        ea = ctx.enter_context(nc.sbuf_tensor("ea", [L, 1], F32))
        sp = ctx.enter_context(nc.sbuf_tensor("sp", [L, 1], F32))
        lt = ctx.enter_context(nc.sbuf_tensor("lt", [L, 1], F32))
        res = ctx.enter_context(nc.sbuf_tensor("res", [1, 1], F32))
        warm = ctx.enter_context(nc.sbuf_tensor("warm", [1, 1], F32))
        ps = ctx.enter_context(nc.psum_tensor("ps", [1, 1], F32))
        w_sem = ctx.enter_context(nc.semaphore("w_sem"))
        g_sem = ctx.enter_context(nc.semaphore("g_sem"))
        ge_sem = ctx.enter_context(nc.semaphore("ge_sem"))
        gb_sem = ctx.enter_context(nc.semaphore("gb_sem"))
        v_sem = ctx.enter_context(nc.semaphore("v_sem"))
        s_sem = ctx.enter_context(nc.semaphore("s_sem"))
        t_sem = ctx.enter_context(nc.semaphore("t_sem"))

        if GATHER_MODE == "dma_gather":
            pa = ctx.enter_context(nc.sbuf_tensor("pa", [128, idx_cols], I32))
            sb16 = ctx.enter_context(nc.sbuf_tensor("sb16", [128, idx_cols], I32))
            shv = ctx.enter_context(nc.sbuf_tensor("shv", [128, idx_cols], I32))
            nv = ctx.enter_context(nc.sbuf_tensor("nv", [128, idx_cols], I32))
            qv = ctx.enter_context(nc.sbuf_tensor("qv", [128, idx_cols], I32))
            idxs16 = ctx.enter_context(nc.sbuf_tensor("idxs16", [128, idx_cols], I16))
            idxs0 = ctx.enter_context(nc.sbuf_tensor("idxs0", [128, idx_cols], I16))
            idxq = ctx.enter_context(
                nc.sbuf_tensor("idxq", [128, 2 * max(NB, 1)], I16)
            )
            gs = ctx.enter_context(nc.sbuf_tensor("gs", [128, 1, EMD_DIM], F32))
            gev = ctx.enter_context(nc.sbuf_tensor("gev", [128, 1, EMD_DIM], F32))
            tmp = ctx.enter_context(nc.sbuf_tensor("tmp", [L, EMD_DIM], F32))

            # ---- preamble: compute the int16 index table on-device ----
            # entry i (= 16*col + partition) of the table must hold
            # path[i] = (leaf1 >> (i+1)) - 1, clamped into [0, 32767]; the
            # first NB entries (the big rows) clamp to 32767 (a dummy row of
            # the 32768-row window) and are replaced by the big-row gathers.
            # The Q7 cores each read their own 16-partition replica of the
            # index table (entry i sits at partition i%16 + 16*core, column
            # i//16), so compute shift[p][s] = (p & 15) + 16*s + 1 on ALL 128
            # partitions. The Pool queue only guarantees ordering 4+ slots
            # back, so each dependent op carries an explicit chain-counter
            # wait; shifts run on DVE (the BIR verifier rejects Pool shifts).
            c_sem = ctx.enter_context(nc.semaphore("c_sem"))
            m_sem = ctx.enter_context(nc.semaphore("m_sem"))
            nc.gpsimd.memset(idxs0[:, :], 0)
            # per-big-row index tables: row = 16*q + c gathered from a
            # 16-row-strided window starting at c, so q fits in int16
            for k in range(NB):
                nc.gpsimd.memset(idxq[:, 2 * k : 2 * k + 2], path[k] // 16)
            nc.gpsimd.memset(warm[:, :], 0.0).then_inc(m_sem, 1)
            nc.gpsimd.iota(
                pa[:, :], pattern=[[0, idx_cols]], base=0, channel_multiplier=1
            ).then_inc(c_sem, 1)
            nc.gpsimd.iota(
                sb16[:, :],
                pattern=[[16, idx_cols]],
                base=NB + 1,
                channel_multiplier=0,
            ).then_inc(c_sem, 1)
            nc.gpsimd.iota(
                nv[:, :], pattern=[[0, idx_cols]], base=leaf1, channel_multiplier=0
            ).then_inc(c_sem, 1)
            nc.vector.tensor_scalar(
                out=pa[:, :],
                in0=pa[:, :],
                scalar1=15,
                op0=mybir.AluOpType.bitwise_and,
                scalar2=0,
                op1=mybir.AluOpType.bitwise_or,
            ).wait_op(c_sem, 2, "sem-ge").then_inc(c_sem, 1)
            nc.vector.tensor_tensor(
                out=shv[:, :],
                in0=pa[:, :],
                in1=sb16[:, :],
                op=mybir.AluOpType.add,
            ).wait_op(c_sem, 4, "sem-ge").then_inc(c_sem, 1)
            nc.vector.tensor_tensor(
                out=qv[:, :],
                in0=nv[:, :],
                in1=shv[:, :],
                op=mybir.AluOpType.arith_shift_right,
            ).wait_op(c_sem, 5, "sem-ge").then_inc(c_sem, 1)
            # entries past the path clamp to -1 (trailing "ignored" marker);
            # the big rows clamp to the window's last (dummy) row; the clamp
            # writes the int16 table directly (int32 -> int16 cast on write)
            nc.vector.tensor_scalar(
                out=idxs16[:, :],
                in0=qv[:, :],
                scalar1=1,
                op0=mybir.AluOpType.subtract,
                scalar2=SMALL_WIN - 1,
                op1=mybir.AluOpType.min,
            ).wait_op(c_sem, 6, "sem-ge").then_inc(w_sem, 1)
        else:
            idxr = ctx.enter_context(nc.sbuf_tensor("idxr", [1, nidx], I32))
            g = ctx.enter_context(nc.sbuf_tensor("g", [nidx, EMD_DIM], F32))
            tmp = ctx.enter_context(nc.sbuf_tensor("tmp", [L, EMD_DIM], F32))
            for k, v in enumerate(path):
                nc.gpsimd.memset(idxr[0:1, k : k + 1], int(v))
            nc.gpsimd.memset(idxr[0:1, L:32], 0)
            nc.gpsimd.memset(idxr[0:1, 32:nidx], int(emd_row)).then_inc(w_sem, 1)

        # ACT table prefetch: one explicit load of the combined Exp+Ln set,
        # emitted before the block streams so it starts at t~0 and hides
        # under the gather phase. (Left to its own devices Bacc's
        # insert_act_table_loads pass picks per-function sets and ends up
        # loading three different tables mid-chain.)
        from concourse.hw_specs import get_activation_tables

        table_names = list(get_activation_tables(nc.m.arch))
        combined_id = table_names.index("natural_log_exp_and_others")
        nc.scalar.add_instruction(
            mybir.InstLoadActFuncSet(
                name=nc.get_next_instruction_name(),
                ins=[],
                outs=[],
                act_func_set_id=combined_id,
            )
        ).then_inc(s_sem, 1)

        block = ctx.enter_context(nc.Block())

        @block.sync
        def _(sync):
            # Final scalar out via sequencer register store: res is 4 bytes,
            # so a TENSOR_LOAD + TENSOR_SAVE replaces a whole DMA.
            sync.wait_ge(s_sem, 5)
            reg = sync.alloc_register("res_out")
            sync.reg_load(reg, res[0:1, 0:1].bitcast(I32))
            sync.store(out[0:1, 0:1].bitcast(I32), reg)

        @block.gpsimd
        def _(gpsimd):
            if GATHER_MODE == "dma_gather":
                # emd row replicated into partitions 0..L-1
                gpsimd.dma_gather(
                    out_ap=gev[:, :, :],
                    in_ap=tbl[emd_row : emd_row + 1, :],
                    idxs_ap=idxs0[:, :],
                    num_idxs=L,
                    num_idxs_reg=L,
                    elem_size=EMD_DIM,
                ).wait_op(m_sem, 1, "sem-ge").then_inc(ge_sem, 16)
                # big path rows into partitions 15+k: each gather floods
                # partitions 0..15+k with row 16*q_k + c_k from a 16-row-
                # strided window; descending chain leaves path[k] at 15+k
                prev = 0
                for k in range(NB - 1, -1, -1):
                    c_k, q_k = path[k] % 16, path[k] // 16
                    ins = gpsimd.dma_gather(
                        out_ap=gs[:, :, :],
                        in_ap=tbl[c_k : c_k + 16 * q_k + 1 : 16, :],
                        idxs_ap=idxq[:, 2 * k : 2 * k + 2 - (16 + k <= 16)],
                        num_idxs=16 + k,
                        num_idxs_reg=16 + k,
                        elem_size=EMD_DIM,
                        elem_step=16 * EMD_DIM,
                    )
                    if prev == 0:
                        ins.wait_op(m_sem, 1, "sem-ge")
                    else:
                        ins.wait_op(gb_sem, prev, "sem-ge")
                    ins.then_inc(gb_sem, 16)
                    prev += 16
                # small path rows overwrite partitions 0..L-NB-1 last
                gpsimd.wait_ge(gb_sem, 16 * NB)
                gpsimd.dma_gather(
                    out_ap=gs[:, :, :],
                    in_ap=tbl[0:SMALL_WIN, :],
                    idxs_ap=idxs16[:, 0 : -(-(L - NB) // 16)],
                    num_idxs=L - NB,
                    num_idxs_reg=L - NB,
                    elem_size=EMD_DIM,
                ).wait_op(w_sem, 1, "sem-ge").then_inc(g_sem, 16)
            else:
                gpsimd.wait_ge(w_sem, 1)
                gpsimd.indirect_dma_start(
                    out=g[:, :],
                    out_offset=None,
                    in_=tbl[:, :],
                    in_offset=bass.IndirectOffsetOnAxis(ap=idxr[0:1, :], axis=0),
                ).then_inc(g_sem, 16)

        @block.vector
        def _(vector):
            # pd[l] = sum_d hv[l,d] * ev[l,d]  — the complete dot products
            if GATHER_MODE == "dma_gather":
                vector.wait_ge(g_sem, 16)
                vector.wait_ge(ge_sem, 16)
                vector.scalar_tensor_tensor(
                    out=tmp[:, :],
                    in0=gs[0:L, 0, :],
                    scalar=1.0,
                    in1=gev[0:L, 0, :],
                    op0=mybir.AluOpType.mult,
                    op1=mybir.AluOpType.mult,
                    accum_out=pd[:, :],
                ).then_inc(v_sem, 2)
            else:
                vector.wait_ge(g_sem, 16)
                vector.scalar_tensor_tensor(
                    out=tmp[:, :],
                    in0=g[0:L, :],
                    scalar=1.0,
                    in1=g[32 : 32 + L, :],
                    op0=mybir.AluOpType.mult,
                    op1=mybir.AluOpType.mult,
                    accum_out=pd[:, :],
                ).then_inc(v_sem, 2)

        @block.scalar
        def _(scalar):
            # sp = softplus(-dots) = log(exp(-dots) + 1) = -log_sigmoid(dots)
            scalar.wait_ge(v_sem, 2)
            scalar.activation(
                ea[:, :],
                pd[:, :],
                mybir.ActivationFunctionType.Exp,
                scale=-1.0,
            ).then_inc(s_sem, 1)
            # ACT pipeline does not forward: same-engine RAW needs waits
            scalar.wait_ge(s_sem, 2)
            scalar.activation(
                sp[:, :],
                ea[:, :],
                mybir.ActivationFunctionType.Ln,
                bias=1.0,
            ).then_inc(s_sem, 1)
            scalar.wait_ge(s_sem, 3)
            scalar.activation(
                lt[:, :],
                sp[:, :],
                mybir.ActivationFunctionType.Ln,
            ).then_inc(s_sem, 1)

            # res = exp(sum_l ln(sp_l)) = prod(sp)
            scalar.wait_ge(t_sem, 1)
            scalar.activation(
                res[:, :],
                ps[:, :],
                mybir.ActivationFunctionType.Exp,
            ).then_inc(s_sem, 1)

        @block.tensor
        def _(tensor):
            # sum over the L partitions: ps = lt.T @ ones
            tensor.wait_ge(s_sem, 4)
            nc.tensor.matmul(
                out=ps[:, :],
                lhsT=lt[:, :],
                rhs=nc.const_aps.tensor(1.0, (L, 1)),
                start=True,
                stop=True,
            ).then_inc(t_sem, 1)

    if not nc.is_finalized():
        nc.finalize()

    # res = prod(sp) = (-1)^L prod(logsig); answer = -prod(logsig), so for odd
    # L the answer is res itself, for even L it is -res (host applies sign).
    sign = 1.0 if L % 2 == 1 else -1.0
    return nc, L, sign


_cache: dict = {}


def _get_module(v_j: int, u_k: int):
    key = (v_j, u_k)
    if key not in _cache:
        _cache[key] = build_module(v_j, u_k)
    return _cache[key]


def shard_inputs(emd_np: np.ndarray, hs_np: np.ndarray, u_k: int, v_j: int = 12345):
    tbl = np.ascontiguousarray(
        np.concatenate([hs_np, emd_np], axis=0, dtype=np.float32)
    )
    return [{"tbl": tbl} for _ in range(N_CORES)]


def kernel(v_j, u_k, emd_weight, hs_weight) -> np.ndarray:
    v_j = int(v_j)
    u_k = int(u_k)
    emd_np = np.asarray(emd_weight, dtype=np.float32)
    hs_np = np.asarray(hs_weight, dtype=np.float32)
    assert emd_np.shape == (NUM_V, EMD_DIM), emd_np.shape
    assert hs_np.shape == (NUM_V - 1, EMD_DIM), hs_np.shape

    nc, L, sign = _get_module(v_j, u_k)
    in_maps = shard_inputs(emd_np, hs_np, u_k, v_j)
    results = run_bass_kernel_spmd(nc, in_maps, list(range(N_CORES))).results
    val = sign * float(results[0]["out"][0, 0])
    return np.float32(val)



# revision 44
# speedup vs baseline: 1.4567x; 1.4567x over previous
"""DeepWalk hierarchical-softmax scoring kernel for 8 Trainium2 NeuronCores.

Computation (mirrors the nn.Module reference):
    path = heap ancestors of leaf u_k           (L ~ 19-20 static ints)
    emd  = emd_weight[v_j]                      [128]
    hv   = hs_weight[path]                      [L, 128]
    out  = -prod(log_sigmoid(hv @ emd))         scalar f32

Distribution: full replication (batch-size-1 degenerate case of the hint's
data parallelism); each core computes the whole scalar, core 0's is returned.
A cross-core collective costs ~15us flat, dwarfing the entire computation.

Two-engine design: Pool fetches, DVE computes; ACT/PE/SP are never touched.
The ACT engine's first table-based activation would charge a 1283ns
ACT_TABLE_LOAD - the entire critical path of the original build - so all
transcendentals are polynomial/bit-trick evaluated on DVE ALUs (walrus
rejects scalar_tensor_tensor, bitwise ops and shifts on Pool, so the
arithmetic cannot ride the gather queue).

  HBM layout (host-staged): one concat table [pad_row; hs; emd] of f32
  rows.  (8-byte-dtype gathers would halve the modeled per-gather cost but
  return garbage on real hardware, so rows move as 128 x f32.)  The pad
  row makes the small-path gather index exactly leaf1 >> s (no -1).

  Fetch (6 SWDGE gathers on Pool, 107ns each, semaphore-chained):
    G_emd: emd row flooded into partitions 0..L-1 (zero idx table)
    G_b(k), k=NB-1..0: big path rows (padded row >= 32768) flooded into
      partitions 0..15+k from single-row windows; descending chain leaves
      path[k] at partition 15+k
    G_small: the L-NB small rows into partitions 0..L-NB-1, idx table
      idx[p] = leaf1 >> min(p+NB+1, 31), built by the DVE preamble
  DVE stt: dot products -> pd[L,1] f32 (194ns; the only non-free op besides
    the final reduce).
  DVE column pipeline (free-size-1 ops cost ~0): u = exp(-x) via 2^(n+f)
    split (trunc-to-int, exponent bits via shift, deg-6 poly for 2^f), then
    ln(softplus(-x)) = ln(u * log1p(u)/u) = -x + lnq(u) with lnq a deg-5
    poly - no mantissa extraction needed since ln(exp(-x)) = -x exactly.
    lnsp lands in column 0 of a [32,32] block pre-zeroed during DVE idle.
  DVE transposing reduce-add (apply_transpose, 94ns): S = sum_l ln sp_l,
    then res = exp(S) via the same 2^(n+f) bit trick (free), and a
    sequencer TENSOR_LOAD/SAVE on the DVE queue stores the 4 bytes
    (no DMACopy, no ACT table, no PE).

Host applies the (-1)^L sign (reference = -prod(logsig) = (-1)^(L+1) prod sp).
"""

import contextlib

import numpy as np

import concourse.bass as bass
import concourse.mybir as mybir
from concourse.bass_utils import run_bass_kernel_spmd

NUM_V = 1_000_000
EMD_DIM = 128
N_CORES = 8
F32 = mybir.dt.float32
I32 = mybir.dt.int32
I16 = mybir.dt.int16
I64 = mybir.dt.int64
W64 = EMD_DIM // 2  # row = 64 x int64 words
TBL_ROWS = 1 + (NUM_V - 1) + NUM_V  # pad + hs + emd
SMALL_WIN = 32768  # int16 index reach of one dma_gather window

LOG2E = 1.4426950408889634
# 2^f on [-0.55, 1.05], deg 6 (rel err 1.2e-7), high -> low; the wide
# range keeps the split u = 2^(n-32) * 2^f correct whether the hardware's
# f32->i32 cast truncates (f in [0,1)) or rounds (f in [-0.5, 0.5])
P2 = [0.0001810241, 0.0013325003, 0.0096054086, 0.0555059128,
      0.2402281463, 0.6931470633, 0.9999999404]
# ln(log1p(u)/u) on [0, 0.42], deg 5 (abs err 1.2e-7), high -> low
LQ = [-0.0273783058, 0.0721771866, -0.1219773367, 0.2080348730,
      -0.4999879599, -0.0000001170]


def hs_path(u_k: int, num_V: int = NUM_V) -> list[int]:
    """Heap indices of all ancestors of leaf u_k, down-to-root (incl. 0)."""
    n = num_V - 1 + u_k
    path = []
    while n > 0:
        n = (n - 1) // 2
        path.append(n)
    return path


def build_module(v_j: int, u_k: int):
    """Build the per-core Bass module. v_j/u_k are compile-time constants,
    mirroring the reference where the path is a static int array."""
    path = hs_path(u_k)
    L = len(path)
    assert L <= 31
    leaf1 = NUM_V + u_k  # (leaf index + 1): path[k] = (leaf1 >> (k+1)) - 1
    # "big" rows: padded row index (path+1) out of int16 gather reach
    NB = sum(1 for p in path if p + 1 >= SMALL_WIN)
    assert all(p + 1 >= SMALL_WIN for p in path[:NB])
    assert all(p + 1 < SMALL_WIN for p in path[NB:])
    assert NB <= 16
    NS = L - NB
    emd_row = 1 + (NUM_V - 1) + v_j  # padded row of emd_weight[v_j]

    # Bacc (vs raw Bass) for its compile passes: InstISA subclass codegen
    # (dma_gather) and automatic GPSIMD library-load insertion.
    from concourse.bacc import Bacc

    nc = Bacc("TRN2", num_devices=N_CORES)

    tbl = nc.dram_tensor("tbl", [TBL_ROWS, EMD_DIM], F32, kind="ExternalInput")
    out = nc.dram_tensor("out", [1, 1], F32, kind="ExternalOutput")

    ctx = contextlib.ExitStack()
    with ctx:
        z16 = ctx.enter_context(nc.sbuf_tensor("z16", [128, 2], I16))
        sv = ctx.enter_context(nc.sbuf_tensor("sv", [128, 1], I32))
        lv = ctx.enter_context(nc.sbuf_tensor("lv", [128, 1], I32))
        svm = ctx.enter_context(nc.sbuf_tensor("svm", [128, 1], I32))
        svc = ctx.enter_context(nc.sbuf_tensor("svc", [128, 1], I32))
        idxw = ctx.enter_context(nc.sbuf_tensor("idxw", [128, 1], I32))
        idx16 = ctx.enter_context(nc.sbuf_tensor("idx16", [128, 1], I16))
        gs = ctx.enter_context(nc.sbuf_tensor("gs", [128, 1, EMD_DIM], F32))
        gev = ctx.enter_context(nc.sbuf_tensor("gev", [128, 1, EMD_DIM], F32))
        tmp = ctx.enter_context(nc.sbuf_tensor("tmp", [L, EMD_DIM], F32))
        pd = ctx.enter_context(nc.sbuf_tensor("pd", [L, 1], F32))
        ya = ctx.enter_context(nc.sbuf_tensor("ya", [L, 1], F32))
        ni = ctx.enter_context(nc.sbuf_tensor("ni", [L, 1], I32))
        nf = ctx.enter_context(nc.sbuf_tensor("nf", [L, 1], F32))
        fr = ctx.enter_context(nc.sbuf_tensor("fr", [L, 1], F32))
        e2 = ctx.enter_context(nc.sbuf_tensor("e2", [L, 1], I32))
        e2a = ctx.enter_context(nc.sbuf_tensor("e2a", [L, 1], I32))
        e2b = ctx.enter_context(nc.sbuf_tensor("e2b", [1, 1], I32))
        pp = ctx.enter_context(nc.sbuf_tensor("pp", [L, 1], F32))
        pq = ctx.enter_context(nc.sbuf_tensor("pq", [L, 1], F32))
        uu = ctx.enter_context(nc.sbuf_tensor("uu", [L, 1], F32))
        lq = ctx.enter_context(nc.sbuf_tensor("lq", [L, 1], F32))
        l2 = ctx.enter_context(nc.sbuf_tensor("l2", [L, 1], F32))
        sp32 = ctx.enter_context(nc.sbuf_tensor("sp32", [32, 32], F32))
        red = ctx.enter_context(nc.sbuf_tensor("red", [32, 1], F32))
        y2 = ctx.enter_context(nc.sbuf_tensor("y2", [1, 1], F32))
        n2 = ctx.enter_context(nc.sbuf_tensor("n2", [1, 1], I32))
        n2f = ctx.enter_context(nc.sbuf_tensor("n2f", [1, 1], F32))
        f2 = ctx.enter_context(nc.sbuf_tensor("f2", [1, 1], F32))
        e22 = ctx.enter_context(nc.sbuf_tensor("e22", [1, 1], I32))
        pa = ctx.enter_context(nc.sbuf_tensor("pa", [1, 1], F32))
        pb = ctx.enter_context(nc.sbuf_tensor("pb", [1, 1], F32))
        res = ctx.enter_context(nc.sbuf_tensor("res", [1, 1], F32))

        m_sem = ctx.enter_context(nc.semaphore("m_sem"))
        i_sem = ctx.enter_context(nc.semaphore("i_sem"))
        w_sem = ctx.enter_context(nc.semaphore("w_sem"))
        gb_sem = ctx.enter_context(nc.semaphore("gb_sem"))
        ge_sem = ctx.enter_context(nc.semaphore("ge_sem"))
        g_sem = ctx.enter_context(nc.semaphore("g_sem"))
        c_sem = ctx.enter_context(nc.semaphore("c_sem"))

        # ---- preamble (Pool from ~t=100; DVE from ~t=200) ----
        nc.gpsimd.memset(z16[:, :], 0).then_inc(m_sem, 1)
        nc.gpsimd.iota(
            lv[:, :], pattern=[[0, 1]], base=leaf1, channel_multiplier=0
        ).then_inc(i_sem, 1)
        nc.gpsimd.iota(
            sv[:, :], pattern=[[0, 1]], base=0, channel_multiplier=1
        ).then_inc(i_sem, 1)
        # small-row idx table: idx[p] = leaf1 >> min((p&15)+NB+1, 31); for
        # p%16 < NS that is exactly (padded) row path[p%16+NB]+1 < 32768.
        # The &15 replicates the table across all 8 Q7 cores' 16-partition
        # views - REAL hardware's cores each read their own replica (the
        # interpreter only reads partitions 0..15, so this is invisible in
        # simulation but mandatory on device). svc's wait covers both iotas.
        nc.vector.tensor_scalar(
            out=svm[:, :], in0=sv[:, :],
            scalar1=15, op0=mybir.AluOpType.bitwise_and,
            scalar2=0, op1=mybir.AluOpType.bitwise_or,
        ).wait_op(i_sem, 2, "sem-ge").then_inc(c_sem, 1)
        nc.vector.tensor_scalar(
            out=svc[:, :], in0=svm[:, :],
            scalar1=NB + 1, op0=mybir.AluOpType.add,
            scalar2=31, op1=mybir.AluOpType.min,
        ).wait_op(c_sem, 1, "sem-ge").then_inc(c_sem, 1)
        nc.vector.tensor_tensor(
            out=idxw[:, :], in0=lv[:, :], in1=svc[:, :],
            op=mybir.AluOpType.arith_shift_right,
        ).wait_op(c_sem, 2, "sem-ge").then_inc(c_sem, 1)
        # i32 -> i16 narrowing on a separate op: the ISA check rejects a
        # shift writing int16 directly
        nc.vector.tensor_scalar(
            out=idx16[:, :], in0=idxw[:, :],
            scalar1=0, op0=mybir.AluOpType.add,
            scalar2=SMALL_WIN - 1, op1=mybir.AluOpType.min,
        ).wait_op(c_sem, 3, "sem-ge").then_inc(w_sem, 1)
        # pad partitions contribute 0.0 to the log-sum; 94ns in the DVE
        # idle window, emitted after idx16 so it can't delay w_sem
        nc.vector.memset(sp32[:, :], 0.0).then_inc(m_sem, 1)

        block = ctx.enter_context(nc.Block())

        @block.gpsimd
        def _(gpsimd):
            # emd row replicated into partitions 0..L-1
            gpsimd.dma_gather(
                out_ap=gev[:, :, :],
                in_ap=tbl[emd_row : emd_row + 1, :],
                idxs_ap=z16[:, 0 : -(-L // 16)],
                num_idxs=L,
                num_idxs_reg=L,
                elem_size=EMD_DIM,
            ).wait_op(m_sem, 1, "sem-ge").then_inc(ge_sem, 16)
            # big path rows: flood partitions 0..15+k from a single-row
            # window; the descending chain leaves path[k] at partition 15+k
            prev = 0
            for k in range(NB - 1, -1, -1):
                ins = gpsimd.dma_gather(
                    out_ap=gs[:, :, :],
                    in_ap=tbl[path[k] + 1 : path[k] + 2, :],
                    idxs_ap=z16[:, 0 : -(-(16 + k) // 16)],
                    num_idxs=16 + k,
                    num_idxs_reg=16 + k,
                    elem_size=EMD_DIM,
                )
                if prev == 0:
                    ins.wait_op(m_sem, 1, "sem-ge")
                else:
                    ins.wait_op(gb_sem, prev, "sem-ge")
                ins.then_inc(gb_sem, 16)
                prev += 16
            # small path rows overwrite partitions 0..NS-1 last
            if prev:
                gpsimd.wait_ge(gb_sem, prev)
            gpsimd.dma_gather(
                out_ap=gs[:, :, :],
                in_ap=tbl[0:SMALL_WIN, :],
                idxs_ap=idx16[:, :],
                num_idxs=NS,
                num_idxs_reg=NS,
                elem_size=EMD_DIM,
            ).wait_op(w_sem, 1, "sem-ge").then_inc(g_sem, 16)

        @block.vector
        def _(vector):
            # Everything after the gathers rides the DVE queue; the [L,1]
            # and [1,1] column ops are free-size-1 and cost ~0 in the
            # model. Same-queue RAW ordering uses a c_sem counter chain
            # (same-engine sem visibility is immediate, so waits are free).
            A = mybir.AluOpType
            cnt = 3  # c_sem is at 3 from the preamble idx chain

            def ts(out_ap, in_ap, s1, op0, s2, op1):
                nonlocal cnt
                ins = vector.tensor_scalar(
                    out=out_ap, in0=in_ap, scalar1=s1, op0=op0,
                    scalar2=s2, op1=op1,
                ).wait_op(c_sem, cnt, "sem-ge")
                cnt += 1
                ins.then_inc(c_sem, 1)
                return ins

            def tt(out_ap, a_ap, b_ap, op):
                nonlocal cnt
                ins = vector.tensor_tensor(
                    out=out_ap, in0=a_ap, in1=b_ap, op=op
                ).wait_op(c_sem, cnt, "sem-ge")
                cnt += 1
                ins.then_inc(c_sem, 1)
                return ins

            # pd[l] = sum_d hv[l,d] * emd[d]
            vector.wait_ge(ge_sem, 16)
            vector.wait_ge(g_sem, 16)
            vector.scalar_tensor_tensor(
                out=tmp[:, :],
                in0=gs[0:L, 0, :],
                scalar=1.0,
                in1=gev[0:L, 0, :],
                op0=A.mult,
                op1=A.mult,
                accum_out=pd[:, :],
            ).wait_op(c_sem, cnt, "sem-ge").then_inc(c_sem, 1)
            cnt += 1

            # u = exp(-x) = 2^(n-32) * 2^f;  ya = 32 - x*log2e = n + f
            ts(ya[:, :], pd[:, :], -LOG2E, A.mult, 32.0, A.add)
            ts(ni[:, :], ya[:, :], 0, A.add, 0, A.bypass)      # trunc
            ts(nf[:, :], ni[:, :], 0, A.add, 0, A.bypass)      # back to f32
            tt(fr[:, :], ya[:, :], nf[:, :], A.subtract)       # f in [0,1)
            # (n+95)<<23 as (n<<23) + (95<<23): walrus rejects mixing a
            # shift with an arith op in one tensor_scalar, so two ops; the
            # add runs in f32 but both addends are multiples of 2^23 < 2^31,
            # so it is exact
            ts(e2a[:, :], ni[:, :], 23, A.arith_shift_left, 0, A.bypass)
            ts(e2[:, :], e2a[:, :], (127 - 32) << 23, A.add, 0, A.bypass)
            # 2^f: Horner deg 5
            ts(pp[:, :], fr[:, :], P2[0], A.mult, P2[1], A.add)
            for c in P2[2:]:
                tt(pq[:, :], pp[:, :], fr[:, :], A.mult)
                ts(pp[:, :], pq[:, :], c, A.add, 0, A.bypass)
            tt(uu[:, :], pp[:, :], e2[:, :].bitcast(F32), A.mult)

            # ln(softplus) = -x + lnq(u), lnq = ln(log1p(u)/u): Horner deg 5
            ts(lq[:, :], uu[:, :], LQ[0], A.mult, LQ[1], A.add)
            for c in LQ[2:]:
                tt(l2[:, :], lq[:, :], uu[:, :], A.mult)
                ts(lq[:, :], l2[:, :], c, A.add, 0, A.bypass)
            tt(sp32[0:L, 0:1], lq[:, :], pd[:, :], A.subtract)

            # S = sum over partitions of column 0: transposing reduce-add
            vector.wait_ge(m_sem, 2)
            vector.tensor_reduce(
                out=red[:, :],
                in_=sp32[:, :],
                axis=mybir.AxisListType.X,
                op=A.add,
                apply_transpose=True,
            ).wait_op(c_sem, cnt, "sem-ge").then_inc(c_sem, 1)
            cnt += 1

            # res = exp(S) = 2^(n2-128) * 2^f2, S in partition 0 of red
            S = red[0:1, 0:1]
            ts(y2[:, :], S, LOG2E, A.mult, 128.0, A.add)
            ts(n2[:, :], y2[:, :], 0, A.add, 0, A.bypass)
            ts(n2f[:, :], n2[:, :], 0, A.add, 0, A.bypass)
            tt(f2[:, :], y2[:, :], n2f[:, :], A.subtract)
            ts(e2b[:, :], n2[:, :], 23, A.arith_shift_left, 0, A.bypass)
            ts(e22[:, :], e2b[:, :], (-1) << 23, A.add, 0, A.bypass)
            ts(pa[:, :], f2[:, :], P2[0], A.mult, P2[1], A.add)
            for c in P2[2:]:
                tt(pb[:, :], pa[:, :], f2[:, :], A.mult)
                ts(pa[:, :], pb[:, :], c, A.add, 0, A.bypass)
            tt(res[:, :], pa[:, :], e22[:, :].bitcast(F32), A.mult)

            # 4-byte result via sequencer register store on this queue
            vector.wait_ge(c_sem, cnt)
            reg = vector.alloc_register("res_out")
            vector.reg_load(reg, res[0:1, 0:1].bitcast(I32))
            vector.store(out[0:1, 0:1].bitcast(I32), reg)

    if not nc.is_finalized():
        nc.finalize()

    # device computes prod(sp) = prod(-logsig); reference = -prod(logsig)
    # = (-1)^(L+1) prod(sp): positive for odd L.
    sign = 1.0 if L % 2 == 1 else -1.0
    return nc, L, sign


_cache: dict = {}


def _get_module(v_j: int, u_k: int):
    key = (v_j, u_k)
    if key not in _cache:
        _cache[key] = build_module(v_j, u_k)
    return _cache[key]


def stage_table(emd_np: np.ndarray, hs_np: np.ndarray) -> np.ndarray:
    """[pad_row; hs; emd] as plain f32 rows (gathers bitcast to i64)."""
    tbl = np.empty((TBL_ROWS, EMD_DIM), dtype=np.float32)
    tbl[0] = 0.0
    tbl[1:NUM_V] = hs_np
    tbl[NUM_V:] = emd_np
    return np.ascontiguousarray(tbl)


def shard_inputs(emd_np: np.ndarray, hs_np: np.ndarray, u_k: int, v_j: int = 12345):
    tbl = stage_table(emd_np, hs_np)
    return [{"tbl": tbl} for _ in range(N_CORES)]


def kernel(v_j, u_k, emd_weight, hs_weight) -> np.ndarray:
    v_j = int(v_j)
    u_k = int(u_k)
    emd_np = np.asarray(emd_weight, dtype=np.float32)
    hs_np = np.asarray(hs_weight, dtype=np.float32)
    assert emd_np.shape == (NUM_V, EMD_DIM), emd_np.shape
    assert hs_np.shape == (NUM_V - 1, EMD_DIM), hs_np.shape

    nc, L, sign = _get_module(v_j, u_k)
    in_maps = shard_inputs(emd_np, hs_np, u_k, v_j)
    results = run_bass_kernel_spmd(nc, in_maps, list(range(N_CORES))).results
    val = sign * float(results[0]["out"][0, 0])
    return np.float32(val)
